# revision 79
# baseline (speedup 1.0000x reference)
"""Multi-head causal self-attention with RoPE on 8 Trainium2 NeuronCores.

Sharding: 12 heads over 8 cores. Core pairs (2p, 2p+1) share 3 heads:
  core 2p:   slot A = head 3p   (all 8 q-blocks), slot B = head 3p+1, q-blocks BSET_EVEN
  core 2p+1: slot A = head 3p+2 (all 8 q-blocks), slot B = head 3p+1, q-blocks BSET_ODD
Every core: 2 heads on 128 partitions, balanced causal cost (both bsets have
equal causal area; the pair splits front-heavy qb0 / tail-heavy qb7 across the
two programs). Two NEFFs dispatched concurrently on device groups [0..3]/[4..7].

v2 (bf16): all matmuls bf16 (same PE rate as fp32r at 512-wide, full rate at
narrow widths). Causal mask folded into the logits PSUM accumulation via an
eye @ M matmul (M = -200 upper triangle; exp(-200*0.125) == 0 for our sums,
and unlike -1e30 it does not NaN the hardware ACT exp table), so no separate
masking pass is needed. AV is restructured as out[q,65] = ex^T @ [V|1] per
(q-tile, k-tile) pair: the scores tile is the stationary operand, so each
accumulation step costs only 65 PE rows instead of 512. NOTE: matmul
start=True clears has_written for the WHOLE PSUM bank, so only the first AV
matmul per slot-bank per q-block sets it. Softmax division is a per-partition
tensor_scalar on DVE (Pool cannot access PSUM) in the [q, dh] layout, then
the result is transposed back to [dh, q] for the O-projection. RoPE pairs are
(d, d+16) within each 32-partition quadrant (host-side weight-row permutation)
so the pair swap is one stream_shuffle rotate-16.

Scheduling (v3): ACT (~0.83 ns/col for the ~13M-element causal exp, ~86us
floor) and PE (~113us busy) are co-binding; the kernel pipelines logits one
group ahead -- emit_logits(g+1) is issued right after emit_exp(g), before any
AV/background PE work, so deferred steps can never delay the logits feeding
the next exp (psL 2-buf rotation provides exactly the right backpressure).
Background work (rope, V-prep, O-projection halves) is queued as cost-tagged
steps and dripped via a per-group PE-slack credit (280ns for 1-slot q-blocks,
600ns for 2-slot), with at most one over-credit "urgent" pop per group so
next-chunk prep never bursts into the exp stream; Q-rope gets an earlier
deadline than K/V-rope since only Q gates a q-block's first logits group.
Front: one contiguous DMA per weight pack (wk | wq | wv+wo | cos+sin fp16 |
eye+mask -- HWDGE costs 625ns per dma_start, so merged), x-chunk-0 split in
half around the wq DMA, and ~3us of dummy matmuls warm the PE pstate
(2.4GHz needs 3us of continuous busy) during the initial DMA wait.
psAV layout note: matmul start=True clears has_written for a WHOLE 2KB PSUM
bank, so the two slots' AV accumulators must sit in different banks
(slot stride AVS=512 f32 cols).
"""
import sys, os
sys.path.insert(0, "/opt/trn_rl_repo")
os.environ.setdefault("MYCRO_LOCAL_CACHE", "1")

import numpy as np

S, D, H, DH = 4096, 768, 12, 64
NCH, CH = 8, 512     # token chunks (projection phase)
NQB, QB = 8, 512     # query blocks
NKT, KT = 32, 128    # key tiles
VPW = 130            # vp_all per-ktile width: [V_A(64) | 1 | V_B(64) | 1]
THETA = 10000.0
ROT16 = [(i + 16) % 32 for i in range(32)]
AVS = 512            # av psum tile: slot s at bank s (start=True clears a whole bank)

import os as _os
_BS = _os.environ.get("MHA_BSETS", "1256")
_ALL = {"1247": ((1, 2, 4, 7), (0, 3, 5, 6)),
        "1346": ((1, 3, 4, 6), (0, 2, 5, 7)),
        "1256": ((1, 2, 5, 6), (0, 3, 4, 7))}
BSET_EVEN, BSET_ODD = _ALL[_BS]

# core -> (headA, headB)
CORE_HEADS = []
for p in range(4):
    CORE_HEADS.append((3 * p, 3 * p + 1))
    CORE_HEADS.append((3 * p + 2, 3 * p + 1))

# row r (0..63) inside a head slot -> original within-head dim.
# quadrant q = r//32, i = r%32: freq f = 16*q + (i%16); i<16 -> dim 2f, else 2f+1.
PERM64 = []
for r in range(64):
    q, i = r // 32, r % 32
    f = 16 * q + (i % 16)
    PERM64.append(2 * f if i < 16 else 2 * f + 1)
PERM64 = np.array(PERM64)

_PROGRAMS = {}


def _build_program(bset):
    import concourse.bass as bass
    import concourse.tile as tile
    from concourse import bacc, mybir
    from concourse.alu_op_type import AluOpType

    dt = mybir.dt
    F32, BF16, F16 = dt.float32, dt.bfloat16, dt.float16
    AF = mybir.ActivationFunctionType

    URG = int(os.environ.get("MHA_URG", "1"))
    SCHED_TRACE = os.environ.get("MHA_SCHED_TRACE") == "1"
    ADD_DVE = os.environ.get("MHA_ADD_DVE", "1") == "1"
    SLACK2 = float(os.environ.get("MHA_SLACK2", "700"))
    SLACK1 = float(os.environ.get("MHA_SLACK1", "280"))
    SLACKL2 = float(os.environ.get("MHA_SLACKL2", "950"))
    SLACKL1 = float(os.environ.get("MHA_SLACKL1", "380"))
    QBLATE = int(os.environ.get("MHA_QBLATE", "5"))

    nc = bacc.Bacc("TRN2", target_bir_lowering=False, debug=False, num_devices=4)

    xt_d = nc.dram_tensor("xt", [D, S], BF16, kind="ExternalInput").ap()
    # weight rows pre-interleaved on host: [128, 6*128] so the DMA is one
    # contiguous full-rate block per weight
    wkt_d = nc.dram_tensor("wkt", [128, D], BF16, kind="ExternalInput").ap()
    wqt_d = nc.dram_tensor("wqt", [128, D], BF16, kind="ExternalInput").ap()
    # wv | wo packed so one DMA covers both
    wvo_d = nc.dram_tensor("wvo", [128, 2 * D], BF16, kind="ExternalInput").ap()
    cossin_d = nc.dram_tensor("cossin", [128, 2 * S], F16, kind="ExternalInput").ap()
    eyemask_d = nc.dram_tensor("eyemask", [128, 256], BF16, kind="ExternalInput").ap()
    opart_d = nc.dram_tensor("opart", [D, S], BF16, kind="ExternalOutput").ap()
    DEBUG = bool(os.environ.get("MHA_DEBUG"))
    if DEBUG:
        dbg_kt = nc.dram_tensor("dbg_kt", [128, S], BF16, kind="ExternalOutput").ap()
        dbg_qt = nc.dram_tensor("dbg_qt", [128, S], BF16, kind="ExternalOutput").ap()
        dbg_vp = nc.dram_tensor("dbg_vp", [128, NKT * VPW], BF16, kind="ExternalOutput").ap()
        dbg_at = nc.dram_tensor("dbg_at", [128, S], BF16, kind="ExternalOutput").ap()

    with tile.TileContext(nc) as tc:
        with (
            tc.tile_pool(name="const", bufs=1) as cp,
            tc.tile_pool(name="xc", bufs=2) as xcp,
            tc.tile_pool(name="rt", bufs=6) as rtp,
            tc.tile_pool(name="ex", bufs=8) as exp_pool,
            tc.tile_pool(name="avs", bufs=10) as avsb,
            tc.tile_pool(name="osb", bufs=3) as osb,
            tc.tile_pool(name="psJ", bufs=2, space="PSUM") as psJ,
            tc.tile_pool(name="psL", bufs=2, space="PSUM") as psL,
            tc.tile_pool(name="psAV", bufs=1, space="PSUM") as psAV,
        ):
            kt_rot = cp.tile([128, S], BF16, tag="ktrot")
            qt_rot = cp.tile([128, S], BF16, tag="qtrot")
            vp_all = cp.tile([128, NKT * VPW], BF16, tag="vpall")
            at_all = cp.tile([128, S], BF16, tag="atall")
            em_all = cp.tile([128, 256], BF16, tag="eyemask")
            eye = em_all[:, 0:128]
            maskm = em_all[:, 128:256]
            wq_all = cp.tile([128, D], BF16, tag="wqa")
            wvo_all = cp.tile([128, 2 * D], BF16, tag="wvo")
            wk_all = cp.tile([128, 6 * 128], BF16, tag="wka")
            ones_sb = cp.tile([128, 64], BF16, tag="ones")
            wq_t = [wq_all[:, i * 128:(i + 1) * 128] for i in range(6)]
            wv_t = [wvo_all[:, i * 128:(i + 1) * 128] for i in range(6)]
            wo_all = wvo_all[:, D:2 * D]
            wk_t = [wk_all[:, i * 128:(i + 1) * 128] for i in range(6)]

            nc.vector.memset(ones_sb[:], 1.0)

            # PE pstate warm-up: ~3us of continuous dummy matmuls during the
            # initial DMA wait so the first real chains run at full clock
            WU = int(os.environ.get("MHA_WU", "7"))
            if WU:
                wu_sb = cp.tile([128, 512], BF16, tag="wu")
                nc.vector.memset(wu_sb[:], 0.0)
                wu_ps = psJ.tile([128, 256], F32, tag="pj", name="wups")
                for _ in range(2 * WU):
                    nc.tensor.matmul(wu_ps[:], wu_sb[:, 0:128],
                                     wu_sb[:, 0:256], start=True, stop=True)

            def init_consts_late():
                # ones columns of vp_all: cols {130t+64, 130t+129}
                nc.vector.tensor_copy(
                    vp_all[:].rearrange(
                        "p (t x) -> p t x", x=VPW)[:, :, 64:VPW:65],
                    ones_sb[:].rearrange("p (t x) -> p t x", x=2))

            nc.sync.dma_start(wk_all[:], wkt_d[:])

            # ---------------- attention -------------------------
            NEVER = NCH + 1  # deadline for steps with no ordering constraint

            oproj_box = {}
            oproj_h0_queued = set()

            def queue_oproj(c, hsel=None):
                contr = 128 if c in bset else 64
                box = oproj_box.setdefault(c, {})

                NOP = 2   # oproj sub-steps per (chunk, mt)
                hw = CH // NOP

                def step(mt, h):
                    # column span [c*CH + h*hw, +hw)
                    if mt == 0 and h == 0:
                        box["ot"] = osb.tile([128, 6 * CH], BF16, tag="ot",
                                             name=f"ot{c}")
                    po = psJ.tile([128, hw], F32, tag="pj",
                                  name=f"po{c}_{mt}_{h}")
                    nc.tensor.matmul(
                        po[:], wo_all[0:contr, mt * 128:(mt + 1) * 128],
                        at_all[0:contr, c * CH + h * hw:c * CH + (h + 1) * hw],
                        start=True, stop=True)
                    # Pool cannot read PSUM; ACT is the binding engine, so
                    # stage on DVE always
                    nc.vector.tensor_copy(
                        box["ot"][:, mt * CH + h * hw:mt * CH + (h + 1) * hw],
                        po[:])
                    if mt == 5 and h == NOP - 1:
                        nc.sync.dma_start(
                            opart_d[:, c * CH:(c + 1) * CH].rearrange(
                                "(mt p) c -> p mt c", p=128),
                            box["ot"][:].rearrange("p (mt c) -> p mt c", c=CH))

                hs = range(NOP) if hsel is None else [hsel]
                for mt in range(6):
                    for h in hs:
                        bg_steps.append(
                            (NEVER, 140.0,
                             (lambda m, hh: lambda: step(m, hh))(mt, h)))

            bg_steps = []  # deferred projection work, drained between groups

            def emit_oproj_span(c, j0, j1, tag2):
                # O-projection over query columns [c*CH + 128*j0, c*CH + 128*j1)
                contr = 128 if c in bset else 64
                w = 128 * (j1 - j0)
                c0 = c * CH + 128 * j0
                ot = osb.tile([128, 6 * w], BF16, tag="ot",
                              name=f"ot{c}_{tag2}")
                half_dma = tag2 == "t3"   # final span: overlap DMA with tail
                for mt in range(6):
                    po = psJ.tile([128, w], F32, tag="pj",
                                  name=f"po{c}_{tag2}_{mt}")
                    nc.tensor.matmul(
                        po[:], wo_all[0:contr, mt * 128:(mt + 1) * 128],
                        at_all[0:contr, c0:c0 + w],
                        start=True, stop=True)
                    if mt % 2 == 0:
                        nc.scalar.copy(ot[:, mt * w:(mt + 1) * w], po[:])
                    else:
                        nc.vector.tensor_copy(ot[:, mt * w:(mt + 1) * w], po[:])
                    if half_dma and mt == 2:
                        nc.sync.dma_start(
                            opart_d[:, c0:c0 + w].rearrange(
                                "(mt p) c -> p mt c", p=128)[:, 0:3],
                            ot[:, 0:3 * w].rearrange(
                                "p (mt c) -> p mt c", c=w))
                if half_dma:
                    nc.sync.dma_start(
                        opart_d[:, c0:c0 + w].rearrange(
                            "(mt p) c -> p mt c", p=128)[:, 3:6],
                        ot[:, 3 * w:6 * w].rearrange(
                            "p (mt c) -> p mt c", c=w))
                else:
                    nc.sync.dma_start(
                        opart_d[:, c0:c0 + w].rearrange(
                            "(mt p) c -> p mt c", p=128),
                        ot[:].rearrange("p (mt c) -> p mt c", c=w))

            def force_need(need):
                # force-drain steps whose chunk the next logits read
                # (scan whole queue: late entries may sit ahead)
                i = 0
                while i < len(bg_steps):
                    if bg_steps[i][0] <= need:
                        bg_steps.pop(i)[2]()
                    else:
                        i += 1

            def emit_logits_for(qb, slots, g, qhalf=None, half_tiles=None):
                ta = 2 * g
                out = {}
                for s in slots:
                    if qhalf is None or qhalf[0] == 0:
                        lg = psL.tile([128, 2 * QB], F32, tag="lg",
                                      name=f"lg{qb}_{g}_{s}")
                        if qhalf is not None:
                            half_tiles[s] = lg
                    else:
                        lg = half_tiles[s]
                    start_col = 0
                    for h in range(2):
                        t = ta + h
                        m = t - 4 * qb
                        off = 128 * m if m >= 0 else 0
                        base = h * QB
                        if h == 0:
                            start_col = off
                        lo = max(off, qhalf[0]) if qhalf else off
                        hi = min(QB, qhalf[1]) if qhalf else QB
                        if lo >= hi:
                            continue
                        nc.tensor.matmul(
                            lg[:, base + lo:base + hi],
                            kt_rot[s * 64:(s + 1) * 64,
                                   t * KT:(t + 1) * KT],
                            qt_rot[s * 64:(s + 1) * 64,
                                   qb * QB + lo:qb * QB + hi],
                            start=True, stop=(m < 0))
                        if m >= 0 and off < hi and off + 128 > lo:
                            # -200 upper-triangle bias via eye @ M
                            nc.tensor.matmul(
                                lg[:, base + off:base + off + 128],
                                eye[:], maskm[:],
                                start=False, stop=True)
                    out[s] = (lg, start_col)
                return out

            def prologue_for(nq):
                # emit qb nq's first logits group (called from qb nq-1's
                # last group so the boundary never gates ACT)
                nslots = [0] + ([1] if nq in bset else [])
                force_need(nq - 0.5)
                return emit_logits_for(nq, nslots, 0)

            def attention_qb(qb, slots, pre=None):
                nkt = 4 * (qb + 1)
                av = psAV.tile([128, 2 * AVS - 252], F32, tag="av",
                               name=f"av{qb}")
                # slot s, qtile j lives at av[:, s*AVS + j*65 : +65]
                avT = {}
                stage2 = []  # (s, j) divisions done, transpose pending

                def finish_stage1(s, j):
                    # reciprocal of the sums column, divide (DVE/Pool)
                    base = s * AVS + j * 65
                    rec = avsb.tile([128, 1], F32, tag="rec",
                                    name=f"rec{qb}_{s}_{j}")
                    with nc.allow_low_precision(reason="softmax recip"):
                        nc.vector.reciprocal(rec[:], av[:, base + 64:base + 65])
                    asb = avsb.tile([128, 64], BF16, tag="asb",
                                    name=f"asb{qb}_{s}_{j}")
                    # Pool cannot read PSUM on trn2 -> divide on DVE
                    nc.vector.tensor_scalar(asb[:], av[:, base:base + 64],
                                            rec[:, 0:1], None,
                                            op0=AluOpType.mult)
                    stage2.append((s, j, asb))

                def finish_stage2(per_j=False):
                    # transpose divided scores tiles back to [dh, q] (PE)
                    flushed = set()
                    for s, j, asb in stage2:
                        flushed.add(j)
                        if per_j:
                            # tail mode: single-qtile transpose + copy so the
                            # final O-projection can start per 128 queries
                            avT1 = psJ.tile([64, 128], BF16, tag="pj",
                                            name=f"avT1{qb}_{s}_{j}")
                            nc.tensor.transpose(avT1[:], asb[:], eye[:])
                            qt0 = 4 * qb + j
                            nc.vector.tensor_copy(
                                at_all[s * 64:(s + 1) * 64,
                                       qt0 * KT:(qt0 + 1) * KT],
                                avT1[:])
                            continue
                        if j % 2 == 0:
                            avT[s] = psJ.tile([64, 256], BF16, tag="pj",
                                              name=f"avT{qb}_{s}_{j}")
                        nc.tensor.transpose(
                            avT[s][:, (j % 2) * 128:(j % 2) * 128 + 128],
                            asb[:], eye[:])
                        if j % 2 == 1:
                            qt0 = 4 * qb + j - 1
                            nc.vector.tensor_copy(
                                at_all[s * 64:(s + 1) * 64,
                                       qt0 * KT:(qt0 + 2) * KT],
                                avT[s][:])
                    stage2.clear()
                    return flushed

                def emit_avs_slot(ex_, ta_, s_):
                    for h_ in range(2):
                        t_ = ta_ + h_
                        for j_ in range(4):
                            qt_ = 4 * qb + j_
                            if t_ > qt_:
                                continue
                            # start=True clears has_written for the WHOLE
                            # PSUM bank on hw, so only the first matmul into
                            # this slot's bank may set it; later slices'
                            # first writes overwrite via the cleared bits
                            nc.tensor.matmul(
                                av[:, s_ * AVS + j_ * 65:
                                   s_ * AVS + j_ * 65 + 65],
                                ex_[:, h_ * QB + j_ * 128:
                                    h_ * QB + j_ * 128 + 128],
                                vp_all[:, t_ * VPW + s_ * 65:
                                       t_ * VPW + s_ * 65 + 65],
                                start=(t_ == 0 and j_ == 0),
                                stop=(t_ == qt_))

                def stage1_checks(ta_):
                    if ta_ + 1 >= 4 * qb:
                        for j_ in (ta_ - 4 * qb, ta_ + 1 - 4 * qb):
                            if 0 <= j_ < 4:
                                for s_ in slots:
                                    finish_stage1(s_, j_)

                def emit_avs(exs_, ta_):
                    for s_ in slots:
                        emit_avs_slot(exs_[s_], ta_, s_)
                    stage1_checks(ta_)

                def emit_logits(g, qhalf=None):
                    return emit_logits_for(qb, slots, g, qhalf, half_tiles)

                def emit_exp(g, lgs_g, qhalf=None):
                    ta = 2 * g
                    exs = {}
                    for s in slots:
                        lg, start_col = lgs_g[s]
                        if qhalf is None or qhalf[0] == 0:
                            ex = exp_pool.tile([128, 2 * QB], BF16, tag="ex",
                                               name=f"ex{qb}_{g}_{s}")
                            if qhalf is not None:
                                half_ex[s] = ex
                        else:
                            ex = half_ex[s]
                        if qhalf is not None:
                            q0, q1 = qhalf
                            # h=0 span [q0:q1]; h=1 valid span starts at 128
                            nc.scalar.activation(ex[:, q0:q1], lg[:, q0:q1],
                                                 AF.Exp, scale=0.125)
                            l1 = max(128, q0)
                            nc.scalar.activation(
                                ex[:, QB + l1:QB + q1],
                                lg[:, QB + l1:QB + q1],
                                AF.Exp, scale=0.125)
                            exs[s] = ex
                            continue
                        m1_ = ta + 1 - 4 * qb
                        if 0 < start_col and m1_ > 1:
                            # second diagonal group: skip the fully-masked
                            # [QB : QB+128*m1_) junk columns with a split exp
                            nc.scalar.activation(ex[:, start_col:QB],
                                                 lg[:, start_col:QB],
                                                 AF.Exp, scale=0.125)
                            nc.scalar.activation(ex[:, QB + 128 * m1_:2 * QB],
                                                 lg[:, QB + 128 * m1_:2 * QB],
                                                 AF.Exp, scale=0.125)
                        else:
                            nc.scalar.activation(ex[:, start_col:2 * QB],
                                                 lg[:, start_col:2 * QB],
                                                 AF.Exp, scale=0.125)
                        exs[s] = ex
                    return exs

                # Pipelined loop: logits for group g+1 are emitted right
                # after exp(g), so background/AV PE work can never delay the
                # logits feeding the next exp.  psL rotation gives exactly
                # the right backpressure (lg g+1 reuses the buf exp(g-ish)
                # reads, and that read precedes in program order).
                ngroups = nkt // 2
                prev = None
                half_tiles = {}
                half_ex = {}
                # per-qb PE slack credit (ns) available for deferred steps
                if qb >= QBLATE:
                    slack_per_group = SLACKL2 if len(slots) == 2 else SLACKL1
                else:
                    slack_per_group = SLACK2 if len(slots) == 2 else SLACK1
                credit = 0.0
                if pre is None:
                    force_need(qb - 0.5)  # only chunk-qb's Q gates group 0
                if qb == 0:
                    # front split: half-A logits+exp only need Q tokens
                    # [0:256) (rope piece a) — ACT starts ~1.5us sooner
                    la = emit_logits(0, qhalf=(0, 256))
                    emit_exp(0, la, qhalf=(0, 256))
                    pend = emit_logits(0, qhalf=(256, 512))
                else:
                    pend = pre if pre is not None else emit_logits(0)
                nxt = None
                for g in range(ngroups):
                    ta = 2 * g
                    if qb == 0 and g == 0:
                        exs = emit_exp(g, pend, qhalf=(256, 512))
                    else:
                        exs = emit_exp(g, pend)
                    pend = None
                    if g + 1 < ngroups:
                        force_need((2 * (g + 1) + 1) // 4)
                        pend = emit_logits(g + 1)
                    elif qb + 1 < NQB:
                        # cross-qb pipelining: next q-block's first logits
                        # are issued under this block's last exp, so the
                        # boundary (final AV/divides) never gates ACT
                        nxt = prologue_for(qb + 1)
                    # AV for the previous group's scores fills PE while ACT
                    # runs the exps.  Force any V-prep whose vp tiles these
                    # AV matmuls read (deadline c+0.75 at most).
                    if prev is not None:
                        tpair = prev[1] + 1
                        force_need(tpair // 4
                                   + (0.75 if tpair % 4 < 2 else 0.9))
                        for s in slots:
                            emit_avs_slot(prev[0][s], prev[1], s)
                    # spend PE slack on deferred steps (cost-budgeted)
                    credit += slack_per_group
                    urgent_used = False
                    while bg_steps:
                        best = min(range(len(bg_steps)),
                                   key=lambda i_: bg_steps[i_][0])
                        dl, cost, fn = bg_steps[best]
                        # at most ONE urgent (over-credit) pop per group so
                        # next-chunk prep never bursts into the exp stream
                        urgent = dl <= qb + URG and not urgent_used
                        if cost <= credit or urgent:
                            bg_steps.pop(best)
                            if SCHED_TRACE:
                                print(f"POP qb{qb} g{g} dl{dl} cost{cost} "
                                      f"urgent{urgent} credit{credit:.0f}")
                            fn()
                            credit -= cost
                            if cost > credit + cost:
                                pass
                            if urgent:
                                urgent_used = True
                                credit = max(credit, 0.0)
                        else:
                            break
                    fl = finish_stage2()
                    if 1 in fl and qb < NQB - 1:
                        # at_all qtiles 4qb..4qb+1 are final: first oproj
                        # half can start dripping within this q-block
                        queue_oproj(qb, hsel=0)
                        oproj_h0_queued.add(qb)
                    if prev is not None:
                        stage1_checks(prev[1])
                    prev = (exs, ta)
                force_need(qb + 0.95)
                emit_avs(*prev)
                if qb == NQB - 1:
                    # tail: interleave the final O-projection with the last
                    # qtiles' division/transpose chains, one qtile at a time
                    rest = list(stage2)
                    for j0 in range(4):
                        stage2[:] = [e for e in rest if e[1] == j0]
                        finish_stage2(per_j=True)
                        emit_oproj_span(qb, j0, j0 + 1, f"t{j0}")
                else:
                    finish_stage2()
                    if qb in oproj_h0_queued:
                        queue_oproj(qb, hsel=1)
                    else:
                        queue_oproj(qb)
                return nxt

            # ---------------- interleaved main loop ---------------------
            def queue_proj_chunk(c):
                """DMA the chunk now; queue K/Q/V proj as background steps."""
                c0, c1 = c * CH, (c + 1) * CH
                xc_all = xcp.tile([128, 6 * CH], BF16, tag="xc", name=f"xca{c}",
                                  bufs=5)
                cs_c = rtp.tile([128, 2 * CH], F16, tag="cosc", name=f"cs{c}",
                                bufs=4)
                cosf_c = cs_c[:, 0:CH]
                sins_c = cs_c[:, CH:2 * CH]
                xr = xc_all[:].rearrange("p (i c) -> p i c", c=CH)
                xtr = xt_d[:, c0:c1].rearrange("(i p) c -> p i c", p=128)
                csr = cs_c[:].rearrange("p (h s) -> p h s", h=2)
                csd = cossin_d[:].rearrange("p (h s) -> p h s", h=2)[:, :, c0:c1]
                if c == 0:
                    # front latency: land the first half-chunk, then wq (Q
                    # projection gate), then cos/sin, then the second half
                    hf = CH // 2
                    nc.sync.dma_start(xr[:, :, 0:hf], xtr[:, :, 0:hf])
                    nc.sync.dma_start(wq_all[:], wqt_d[:])
                    nc.sync.dma_start(csr, csd)
                    # eye+mask before x0b: the half-A logits' mask matmuls
                    # need it ~1us before K-b (x0b's consumer) is read
                    nc.sync.dma_start(em_all[:], eyemask_d[:])
                    nc.sync.dma_start(xr[:, :, hf:CH], xtr[:, :, hf:CH])
                else:
                    nc.sync.dma_start(xr, xtr)
                    nc.sync.dma_start(csr, csd)
                xc = [xc_all[:, i * CH:(i + 1) * CH] for i in range(6)]

                def rope_piece(w, dst, p0, p1):
                    # rope over token columns [c0+p0, c0+p1) of this chunk
                    pw = p1 - p0
                    ps = psJ.tile([128, pw], F32, tag="pj", name=f"pp{c}_{p0}")
                    for i in range(6):
                        nc.tensor.matmul(ps[:], w[i], xc[i][:, p0:p1],
                                         start=(i == 0), stop=(i == 5))
                    tsw = rtp.tile([128, pw], F32, tag="tsw")
                    nc.vector.stream_shuffle(tsw[:], ps[:], ROT16)
                    m1 = rtp.tile([128, pw], F32, tag="m1")
                    nc.vector.tensor_tensor(m1[:], ps[:], cosf_c[:, p0:p1],
                                            op=AluOpType.mult)
                    m2 = rtp.tile([128, pw], F32, tag="m2")
                    nc.gpsimd.tensor_tensor(m2[:], tsw[:], sins_c[:, p0:p1],
                                            op=AluOpType.mult)
                    # add on DVE: Pool's 2x-slower ops would serialize the
                    # rope tail on the critical path to the next logits
                    aeng = nc.vector if (ADD_DVE or c <= 1) else nc.gpsimd
                    aeng.tensor_tensor(dst[:, c0 + p0:c0 + p1],
                                       m1[:], m2[:], op=AluOpType.add)

                def rope_step(w, dst, pieces=1):
                    pw = CH // pieces
                    for p in range(pieces):
                        rope_piece(w, dst, p * pw, (p + 1) * pw)

                def v_tile_step(i_):
                    # V^T-direct: out [tok 128, dh 128] per token tile —
                    # stationary x slice gives token output partitions, so V
                    # lands already transposed for the AV moving operand (no
                    # separate PE transpose or vt staging copy)
                    t_ = 4 * c + i_
                    vtp = psJ.tile([128, 128], F32, tag="pj",
                                   name=f"vtp{c}_{i_}")
                    for i in range(6):
                        nc.tensor.matmul(
                            vtp[:], xc[i][:, i_ * 128:(i_ + 1) * 128],
                            wv_t[i], start=(i == 0), stop=(i == 5))
                    nc.vector.tensor_copy(
                        vp_all[:, t_ * VPW:t_ * VPW + 64], vtp[:, 0:64])
                    nc.vector.tensor_copy(
                        vp_all[:, t_ * VPW + 65:t_ * VPW + 129],
                        vtp[:, 64:128])

                pieces = 2
                pw = CH // pieces
                rp_cost = pw * 2.6
                if c == 0:
                    # qb0 g0 needs kt tiles 0-1 (K-a) + ALL of Q; K-b (tiles
                    # 2-3) is only read at g1 -> order K-a, Q-a, Q-b, K-b
                    order = [(wk_t, kt_rot, 0), (wq_t, qt_rot, 0)]
                    for p in range(1, pieces):
                        order.append((wq_t, qt_rot, p * pw))
                    for p in range(1, pieces):
                        order.append((wk_t, kt_rot, p * pw))
                    for w_, dst_, p0_ in order:
                        bg_steps.append((c, rp_cost,
                                         (lambda w2, d2, p0: lambda:
                                          rope_piece(w2, d2, p0, p0 + pw))
                                         (w_, dst_, p0_)))
                else:
                    # Q-rope gates qb c's FIRST logits group (K/V tiles of
                    # chunk c are only read late in qb c) -> Q first, with an
                    # earlier deadline so drips/forces prioritise it
                    for p in range(pieces):
                        bg_steps.append((c - 0.5, rp_cost, (lambda p0: lambda:
                                         rope_piece(wq_t, qt_rot, p0,
                                                    p0 + pw))(p * pw)))
                    for p in range(pieces):
                        bg_steps.append((c, rp_cost, (lambda p0: lambda:
                                         rope_piece(wk_t, kt_rot, p0,
                                                    p0 + pw))(p * pw)))
                # V tile i_ is first read by the AV pair covering k-tiles
                # 4c+2*(i_//2) -> deadline c+0.7 (first pair) / c+0.9 (second)
                for i_ in range(4):
                    vdl = (0.0 if c == 0 else c) + (0.7 if i_ < 2 else 0.85)
                    bg_steps.append((vdl, 380.0, (lambda ii: lambda:
                                     v_tile_step(ii))(i_)))

            def drain_bg():
                while bg_steps:
                    bg_steps.pop(0)[2]()

            queue_proj_chunk(0)
            init_consts_late()
            # drain chunk 0 now; later chunks prefetch 2 ahead and their
            # compute interleaves into the attention groups
            i0 = 0
            while i0 < len(bg_steps):
                if bg_steps[i0][0] <= 0:
                    bg_steps.pop(i0)[2]()
                else:
                    i0 += 1
            queue_proj_chunk(1)
            # wv/wo after chunk-1's x/cos DMAs: x1 gates qb1's Q-rope (ACT
            # critical path) while wv is first read ~1.5us later by V-prep
            nc.sync.dma_start(wvo_all[:], wvo_d[:])
            nxt_pre = None
            for qb in range(NQB):
                if qb == 0:
                    queue_proj_chunk(2)
                if qb + 3 < NCH:
                    queue_proj_chunk(qb + 3)
                nxt_pre = attention_qb(qb, [0] + ([1] if qb in bset else []),
                                       pre=nxt_pre)
            drain_bg()
            if DEBUG:
                nc.sync.dma_start(dbg_kt[:], kt_rot[:])
                nc.sync.dma_start(dbg_qt[:], qt_rot[:])
                nc.sync.dma_start(dbg_vp[:], vp_all[:])
                nc.sync.dma_start(dbg_at[:], at_all[:])

    nc.compile()
    return nc


def _get_program(bset):
    key = tuple(bset)
    if key not in _PROGRAMS:
        _PROGRAMS[key] = _build_program(key)
    return _PROGRAMS[key]


def _to_bf16(a):
    import ml_dtypes
    return np.asarray(a, np.float32).astype(ml_dtypes.bfloat16)


def _prep_core_inputs(core, x2d_T16, token_positions, Wq, Wk, Wv, Wo):
    hA, hB = CORE_HEADS[core]
    pos = token_positions.astype(np.float64)
    inv_freq = 1.0 / (THETA ** (np.arange(0, DH, 2, dtype=np.float64) / DH))  # [32]
    ang = pos[:, None] * inv_freq[None, :]          # [S, 32]
    cosv, sinv = np.cos(ang), np.sin(ang)           # [S, 32]

    cosf = np.empty((128, S), np.float16)
    sins = np.empty((128, S), np.float16)
    for r in range(64):
        q, i = r // 32, r % 32
        f = 16 * q + (i % 16)
        cosf[r] = cosf[r + 64] = cosv[:, f].astype(np.float16)
        sgn = -1.0 if i < 16 else 1.0
        sins[r] = sins[r + 64] = (sgn * sinv[:, f]).astype(np.float16)

    def _winterleave(wt):
        # [768, 128] -> [128, 6*128]: partition p holds rows {128i+p}
        return np.ascontiguousarray(
            wt.reshape(6, 128, 128).transpose(1, 0, 2).reshape(128, 768))

    rows = np.concatenate([hA * DH + PERM64, hB * DH + PERM64])
    wqt = _to_bf16(_winterleave(Wq[rows].T))   # [128,768]
    wkt = _to_bf16(_winterleave(Wk[rows].T))
    vrows = np.concatenate([np.arange(hA * DH, (hA + 1) * DH),
                            np.arange(hB * DH, (hB + 1) * DH)])
    wvt = _to_bf16(_winterleave(Wv[vrows].T))  # [128,768]
    wot = _to_bf16(np.ascontiguousarray(Wo[:, vrows].T))  # [128,768]

    # -200 (not -inf): exp(-200*0.125) ~ 1e-11 == 0 for our sums, and the
    # hardware ACT exp table NaNs on astronomically negative inputs
    maskm = np.where(np.arange(128)[None, :] >= np.arange(128)[:, None],
                     0.0, -200.0).astype(np.float32)  # [k', q']
    return {
        "xt": x2d_T16,
        "wkt": wkt,
        "wqt": wqt,
        "wvo": np.concatenate([wvt, wot], axis=1),
        "cossin": np.concatenate([cosf, sins], axis=1),
        "eyemask": np.concatenate(
            [_to_bf16(np.eye(128, dtype=np.float32)), _to_bf16(maskm)],
            axis=1),
    }


def _dispatch_group(nc, in_maps, devices):
    """Async-dispatch one program on a device subset; returns (arrs, names, avals, n)."""
    import jax
    from jax.sharding import Mesh, PartitionSpec
    from concourse import bass2jax, mybir

    bass2jax.install_neuronx_cc_hook()
    n = len(in_maps)
    partition_name = (nc.partition_id_tensor.name
                      if nc.partition_id_tensor else None)
    in_names, out_names, out_avals, zero_outs = [], [], [], []
    for alloc in nc.m.functions[0].allocations:
        if not isinstance(alloc, mybir.MemoryLocationSet):
            continue
        name = alloc.memorylocations[0].name
        if alloc.kind == "ExternalInput":
            if name != partition_name:
                in_names.append(name)
        elif alloc.kind == "ExternalOutput":
            shape = tuple(alloc.tensor_shape)
            dtype = mybir.dt.np(alloc.dtype)
            out_names.append(name)
            out_avals.append(jax.core.ShapedArray(shape, dtype))
            zero_outs.append(np.zeros(shape, dtype))
    n_params = len(in_names)
    all_names = in_names + out_names
    if partition_name is not None:
        all_names = all_names + [partition_name]
    donate = tuple(range(n_params, n_params + len(out_names)))

    def _body(*args):
        operands = list(args)
        if partition_name is not None:
            operands.append(bass2jax.partition_id_tensor())
        outs = bass2jax._bass_exec_p.bind(
            *operands, out_avals=tuple(out_avals), in_names=tuple(all_names),
            out_names=tuple(out_names), lowering_input_output_aliases=(),
            sim_require_finite=True, sim_require_nnan=True, nc=nc)
        return tuple(outs)

    try:
        from jax.experimental.shard_map import shard_map
    except ImportError:
        from jax.shard_map import shard_map  # newer jax

    mesh = Mesh(np.asarray(devices), ("core",))
    in_specs = (PartitionSpec("core"),) * (n_params + len(out_names))
    out_specs = (PartitionSpec("core"),) * len(out_names)
    sharded = jax.jit(
        shard_map(_body, mesh=mesh, in_specs=in_specs, out_specs=out_specs,
                  check_rep=False),
        donate_argnums=donate, keep_unused=True)
    per_core = [[np.asarray(m[nm]) for nm in in_names] for m in in_maps]
    concat_in = [np.concatenate([per_core[c][i] for c in range(n)], axis=0)
                 for i in range(n_params)]
    concat_zeros = [np.zeros((n * z.shape[0], *z.shape[1:]), z.dtype)
                    for z in zero_outs]
    out_arrs = sharded(*concat_in, *concat_zeros)
    return out_arrs, out_names, out_avals, n


def kernel(x, token_positions, Wq, Wk, Wv, Wo):
    import jax

    x = np.asarray(x)
    token_positions = np.asarray(token_positions)
    Wq, Wk, Wv, Wo = (np.asarray(a, np.float32) for a in (Wq, Wk, Wv, Wo))
    B = x.shape[0]
    assert x.shape == (B, S, D) and B == 1

    x2d_T16 = _to_bf16(np.ascontiguousarray(x[0].T))  # [768, 4096] bf16

    in_maps = [_prep_core_inputs(c, x2d_T16, token_positions, Wq, Wk, Wv, Wo)
               for c in range(8)]

    nc_even = _get_program(BSET_EVEN)
    nc_odd = _get_program(BSET_ODD)

    devs = jax.devices()
    # even program on devices 0-3 <- logical cores 0,2,4,6
    # odd  program on devices 4-7 <- logical cores 1,3,5,7
    g1_maps = [in_maps[c] for c in (0, 2, 4, 6)]
    g2_maps = [in_maps[c] for c in (1, 3, 5, 7)]

    arrs1, names1, avals1, n1 = _dispatch_group(nc_even, g1_maps, devs[0:4])
    arrs2, names2, avals2, n2 = _dispatch_group(nc_odd, g2_maps, devs[4:8])

    def collect(arrs, names, avals, n):
        res = []
        for c in range(n):
            res.append({
                nm: np.asarray(arrs[i]).reshape(n, *avals[i].shape)[c]
                for i, nm in enumerate(names)})
        return res

    res1 = collect(arrs1, names1, avals1, n1)
    res2 = collect(arrs2, names2, avals2, n2)

    acc = np.zeros((D, S), np.float32)
    for r in res1 + res2:
        acc += r["opart"].astype(np.float32)
    out = np.ascontiguousarray(acc.T).reshape(1, S, D)
    return out



# revision 84
# speedup vs baseline: 1.0026x; 1.0026x over previous
"""Multi-head causal self-attention with RoPE on 8 Trainium2 NeuronCores.

Sharding: 12 heads over 8 cores. Core pairs (2p, 2p+1) share 3 heads:
  core 2p:   slot A = head 3p   (all 8 q-blocks), slot B = head 3p+1, q-blocks BSET_EVEN
  core 2p+1: slot A = head 3p+2 (all 8 q-blocks), slot B = head 3p+1, q-blocks BSET_ODD
Every core: 2 heads on 128 partitions, balanced causal cost (both bsets have
equal causal area; the pair splits front-heavy qb0 / tail-heavy qb7 across the
two programs). Two NEFFs dispatched concurrently on device groups [0..3]/[4..7].

v2 (bf16): all matmuls bf16 (same PE rate as fp32r at 512-wide, full rate at
narrow widths). Causal mask folded into the logits PSUM accumulation via an
eye @ M matmul (M = -200 upper triangle; exp(-200*0.125) == 0 for our sums,
and unlike -1e30 it does not NaN the hardware ACT exp table), so no separate
masking pass is needed. AV is restructured as out[q,65] = ex^T @ [V|1] per
(q-tile, k-tile) pair: the scores tile is the stationary operand, so each
accumulation step costs only 65 PE rows instead of 512. NOTE: matmul
start=True clears has_written for the WHOLE PSUM bank, so only the first AV
matmul per slot-bank per q-block sets it. Softmax division is a per-partition
tensor_scalar on DVE (Pool cannot access PSUM) in the [q, dh] layout, then
the result is transposed back to [dh, q] for the O-projection. RoPE pairs are
(d, d+16) within each 32-partition quadrant (host-side weight-row permutation)
so the pair swap is one stream_shuffle rotate-16.

Scheduling (v3): ACT (~0.83 ns/col for the ~13M-element causal exp, ~86us
floor) and PE (~113us busy) are co-binding; the kernel pipelines logits one
group ahead -- emit_logits(g+1) is issued right after emit_exp(g), before any
AV/background PE work, so deferred steps can never delay the logits feeding
the next exp (psL 2-buf rotation provides exactly the right backpressure).
Background work (rope, V-prep, O-projection halves) is queued as cost-tagged
steps and dripped via a per-group PE-slack credit (280ns for 1-slot q-blocks,
600ns for 2-slot), with at most one over-credit "urgent" pop per group so
next-chunk prep never bursts into the exp stream; Q-rope gets an earlier
deadline than K/V-rope since only Q gates a q-block's first logits group.
Front: one contiguous DMA per weight pack (wk | wq | wv+wo | cos+sin fp16 |
eye+mask -- HWDGE costs 625ns per dma_start, so merged), x-chunk-0 split in
half around the wq DMA, and ~3us of dummy matmuls warm the PE pstate
(2.4GHz needs 3us of continuous busy) during the initial DMA wait.
psAV layout note: matmul start=True clears has_written for a WHOLE 2KB PSUM
bank, so the two slots' AV accumulators must sit in different banks
(slot stride AVS=512 f32 cols).
"""
import sys, os
sys.path.insert(0, "/opt/trn_rl_repo")
os.environ.setdefault("MYCRO_LOCAL_CACHE", "1")

import numpy as np

S, D, H, DH = 4096, 768, 12, 64
NCH, CH = 8, 512     # token chunks (projection phase)
NQB, QB = 8, 512     # query blocks
NKT, KT = 32, 128    # key tiles
VPW = 130            # vp_all per-ktile width: [V_A(64) | 1 | V_B(64) | 1]
THETA = 10000.0
ROT16 = [(i + 16) % 32 for i in range(32)]
AVS = 512            # av psum tile: slot s at bank s (start=True clears a whole bank)

import os as _os
_BS = _os.environ.get("MHA_BSETS", "1256")
_ALL = {"1247": ((1, 2, 4, 7), (0, 3, 5, 6)),
        "1346": ((1, 3, 4, 6), (0, 2, 5, 7)),
        "1256": ((1, 2, 5, 6), (0, 3, 4, 7))}
BSET_EVEN, BSET_ODD = _ALL[_BS]

# core -> (headA, headB)
CORE_HEADS = []
for p in range(4):
    CORE_HEADS.append((3 * p, 3 * p + 1))
    CORE_HEADS.append((3 * p + 2, 3 * p + 1))

# row r (0..63) inside a head slot -> original within-head dim.
# quadrant q = r//32, i = r%32: freq f = 16*q + (i%16); i<16 -> dim 2f, else 2f+1.
PERM64 = []
for r in range(64):
    q, i = r // 32, r % 32
    f = 16 * q + (i % 16)
    PERM64.append(2 * f if i < 16 else 2 * f + 1)
PERM64 = np.array(PERM64)

_PROGRAMS = {}


def _build_program(bset):
    import concourse.bass as bass
    import concourse.tile as tile
    from concourse import bacc, mybir
    from concourse.alu_op_type import AluOpType

    dt = mybir.dt
    F32, BF16, F16 = dt.float32, dt.bfloat16, dt.float16
    AF = mybir.ActivationFunctionType

    URG = int(os.environ.get("MHA_URG", "1"))
    SCHED_TRACE = os.environ.get("MHA_SCHED_TRACE") == "1"
    ADD_DVE = os.environ.get("MHA_ADD_DVE", "1") == "1"
    SLACK2 = float(os.environ.get("MHA_SLACK2", "700"))
    SLACK1 = float(os.environ.get("MHA_SLACK1", "280"))
    SLACKL2 = float(os.environ.get("MHA_SLACKL2", "950"))
    SLACKL1 = float(os.environ.get("MHA_SLACKL1", "380"))
    QBLATE = int(os.environ.get("MHA_QBLATE", "5"))

    nc = bacc.Bacc("TRN2", target_bir_lowering=False, debug=False, num_devices=4)

    xt_d = nc.dram_tensor("xt", [D, S], BF16, kind="ExternalInput").ap()
    # weight rows pre-interleaved on host: [128, 6*128] so the DMA is one
    # contiguous full-rate block per weight
    wkt_d = nc.dram_tensor("wkt", [128, D], BF16, kind="ExternalInput").ap()
    wqt_d = nc.dram_tensor("wqt", [128, D], BF16, kind="ExternalInput").ap()
    # wv | wo packed so one DMA covers both
    wvo_d = nc.dram_tensor("wvo", [128, 2 * D], BF16, kind="ExternalInput").ap()
    cossin_d = nc.dram_tensor("cossin", [128, 2 * S], F16, kind="ExternalInput").ap()
    eyemask_d = nc.dram_tensor("eyemask", [128, 256], BF16, kind="ExternalInput").ap()
    opart_d = nc.dram_tensor("opart", [D, S], BF16, kind="ExternalOutput").ap()
    DEBUG = bool(os.environ.get("MHA_DEBUG"))
    if DEBUG:
        dbg_kt = nc.dram_tensor("dbg_kt", [128, S], BF16, kind="ExternalOutput").ap()
        dbg_qt = nc.dram_tensor("dbg_qt", [128, S], BF16, kind="ExternalOutput").ap()
        dbg_vp = nc.dram_tensor("dbg_vp", [128, NKT * VPW], BF16, kind="ExternalOutput").ap()
        dbg_at = nc.dram_tensor("dbg_at", [128, S], BF16, kind="ExternalOutput").ap()

    with tile.TileContext(nc) as tc:
        with (
            tc.tile_pool(name="const", bufs=1) as cp,
            tc.tile_pool(name="xc", bufs=2) as xcp,  # (xc tile uses bufs=5 override)
            tc.tile_pool(name="rt", bufs=8) as rtp,
            tc.tile_pool(name="ex", bufs=16) as exp_pool,
            tc.tile_pool(name="avs", bufs=20) as avsb,
            tc.tile_pool(name="osb", bufs=4) as osb,
            tc.tile_pool(name="psJ", bufs=2, space="PSUM") as psJ,
            tc.tile_pool(name="psL", bufs=2, space="PSUM") as psL,
            tc.tile_pool(name="psAV", bufs=1, space="PSUM") as psAV,
        ):
            kt_rot = cp.tile([128, S], BF16, tag="ktrot")
            qt_rot = cp.tile([128, S], BF16, tag="qtrot")
            vp_all = cp.tile([128, NKT * VPW], BF16, tag="vpall")
            at_all = cp.tile([128, S], BF16, tag="atall")
            em_all = cp.tile([128, 256], BF16, tag="eyemask")
            eye = em_all[:, 0:128]
            maskm = em_all[:, 128:256]
            wq_all = cp.tile([128, D], BF16, tag="wqa")
            wvo_all = cp.tile([128, 2 * D], BF16, tag="wvo")
            wk_all = cp.tile([128, 6 * 128], BF16, tag="wka")
            ones_sb = cp.tile([128, 64], BF16, tag="ones")
            wq_t = [wq_all[:, i * 128:(i + 1) * 128] for i in range(6)]
            wv_t = [wvo_all[:, i * 128:(i + 1) * 128] for i in range(6)]
            wo_all = wvo_all[:, D:2 * D]
            wk_t = [wk_all[:, i * 128:(i + 1) * 128] for i in range(6)]

            nc.vector.memset(ones_sb[:], 1.0)

            # PE pstate warm-up: ~3us of continuous dummy matmuls during the
            # initial DMA wait so the first real chains run at full clock
            WU = int(os.environ.get("MHA_WU", "7"))
            if WU:
                wu_sb = cp.tile([128, 512], BF16, tag="wu")
                nc.vector.memset(wu_sb[:], 0.0)
                wu_ps = psJ.tile([128, 256], F32, tag="pj", name="wups")
                for _ in range(2 * WU):
                    nc.tensor.matmul(wu_ps[:], wu_sb[:, 0:128],
                                     wu_sb[:, 0:256], start=True, stop=True)

            def init_consts_late():
                # ones columns of vp_all: cols {130t+64, 130t+129}
                nc.vector.tensor_copy(
                    vp_all[:].rearrange(
                        "p (t x) -> p t x", x=VPW)[:, :, 64:VPW:65],
                    ones_sb[:].rearrange("p (t x) -> p t x", x=2))

            nc.sync.dma_start(wk_all[:], wkt_d[:])

            # ---------------- attention -------------------------
            NEVER = NCH + 1  # deadline for steps with no ordering constraint

            oproj_box = {}
            oproj_h0_queued = set()

            def queue_oproj(c, hsel=None):
                contr = 128 if c in bset else 64
                box = oproj_box.setdefault(c, {})

                NOP = 2   # oproj sub-steps per (chunk, mt)
                hw = CH // NOP

                def step(mt, h):
                    # column span [c*CH + h*hw, +hw)
                    if mt == 0 and h == 0:
                        box["ot"] = osb.tile([128, 6 * CH], BF16, tag="ot",
                                             name=f"ot{c}")
                    po = psJ.tile([128, hw], F32, tag="pj",
                                  name=f"po{c}_{mt}_{h}")
                    nc.tensor.matmul(
                        po[:], wo_all[0:contr, mt * 128:(mt + 1) * 128],
                        at_all[0:contr, c * CH + h * hw:c * CH + (h + 1) * hw],
                        start=True, stop=True)
                    # Pool cannot read PSUM; ACT is the binding engine, so
                    # stage on DVE always
                    nc.vector.tensor_copy(
                        box["ot"][:, mt * CH + h * hw:mt * CH + (h + 1) * hw],
                        po[:])
                    if mt == 5 and h == NOP - 1:
                        nc.sync.dma_start(
                            opart_d[:, c * CH:(c + 1) * CH].rearrange(
                                "(mt p) c -> p mt c", p=128),
                            box["ot"][:].rearrange("p (mt c) -> p mt c", c=CH))

                hs = range(NOP) if hsel is None else [hsel]
                for mt in range(6):
                    for h in hs:
                        bg_steps.append(
                            (NEVER, 140.0,
                             (lambda m, hh: lambda: step(m, hh))(mt, h)))

            bg_steps = []  # deferred projection work, drained between groups

            def emit_oproj_span(c, j0, j1, tag2):
                # O-projection over query columns [c*CH + 128*j0, c*CH + 128*j1)
                contr = 128 if c in bset else 64
                w = 128 * (j1 - j0)
                c0 = c * CH + 128 * j0
                ot = osb.tile([128, 6 * w], BF16, tag="ot",
                              name=f"ot{c}_{tag2}")
                half_dma = tag2 == "t3"   # final span: overlap DMA with tail
                for mt in range(6):
                    po = psJ.tile([128, w], F32, tag="pj",
                                  name=f"po{c}_{tag2}_{mt}")
                    nc.tensor.matmul(
                        po[:], wo_all[0:contr, mt * 128:(mt + 1) * 128],
                        at_all[0:contr, c0:c0 + w],
                        start=True, stop=True)
                    if mt % 2 == 0:
                        nc.scalar.copy(ot[:, mt * w:(mt + 1) * w], po[:])
                    else:
                        nc.vector.tensor_copy(ot[:, mt * w:(mt + 1) * w], po[:])
                    if half_dma and mt == 2:
                        nc.sync.dma_start(
                            opart_d[:, c0:c0 + w].rearrange(
                                "(mt p) c -> p mt c", p=128)[:, 0:3],
                            ot[:, 0:3 * w].rearrange(
                                "p (mt c) -> p mt c", c=w))
                if half_dma:
                    nc.sync.dma_start(
                        opart_d[:, c0:c0 + w].rearrange(
                            "(mt p) c -> p mt c", p=128)[:, 3:6],
                        ot[:, 3 * w:6 * w].rearrange(
                            "p (mt c) -> p mt c", c=w))
                else:
                    nc.sync.dma_start(
                        opart_d[:, c0:c0 + w].rearrange(
                            "(mt p) c -> p mt c", p=128),
                        ot[:].rearrange("p (mt c) -> p mt c", c=w))

            def force_need(need):
                # force-drain steps whose chunk the next logits read
                # (scan whole queue: late entries may sit ahead)
                i = 0
                while i < len(bg_steps):
                    if bg_steps[i][0] <= need:
                        bg_steps.pop(i)[2]()
                    else:
                        i += 1

            def emit_logits_for(qb, slots, g, qhalf=None, half_tiles=None):
                ta = 2 * g
                out = {}
                for s in slots:
                    if qhalf is None or qhalf[0] == 0:
                        lg = psL.tile([128, 2 * QB], F32, tag="lg",
                                      name=f"lg{qb}_{g}_{s}")
                        if qhalf is not None:
                            half_tiles[s] = lg
                    else:
                        lg = half_tiles[s]
                    start_col = 0
                    for h in range(2):
                        t = ta + h
                        m = t - 4 * qb
                        off = 128 * m if m >= 0 else 0
                        base = h * QB
                        if h == 0:
                            start_col = off
                        lo = max(off, qhalf[0]) if qhalf else off
                        hi = min(QB, qhalf[1]) if qhalf else QB
                        if lo >= hi:
                            continue
                        nc.tensor.matmul(
                            lg[:, base + lo:base + hi],
                            kt_rot[s * 64:(s + 1) * 64,
                                   t * KT:(t + 1) * KT],
                            qt_rot[s * 64:(s + 1) * 64,
                                   qb * QB + lo:qb * QB + hi],
                            start=True, stop=(m < 0))
                        if m >= 0 and off < hi and off + 128 > lo:
                            # -200 upper-triangle bias via eye @ M
                            nc.tensor.matmul(
                                lg[:, base + off:base + off + 128],
                                eye[:], maskm[:],
                                start=False, stop=True)
                    out[s] = (lg, start_col)
                return out

            def prologue_for(nq):
                # emit qb nq's first logits group (called from qb nq-1's
                # last group so the boundary never gates ACT)
                nslots = [0] + ([1] if nq in bset else [])
                force_need(nq - 0.5)
                return emit_logits_for(nq, nslots, 0)

            def attention_qb(qb, slots, pre=None):
                nkt = 4 * (qb + 1)
                av = psAV.tile([128, 2 * AVS - 252], F32, tag="av",
                               name=f"av{qb}")
                # slot s, qtile j lives at av[:, s*AVS + j*65 : +65]
                avT = {}
                stage2 = []  # (s, j) divisions done, transpose pending

                def finish_stage1(s, j):
                    # reciprocal of the sums column, divide (DVE/Pool)
                    base = s * AVS + j * 65
                    rec = avsb.tile([128, 1], F32, tag="rec",
                                    name=f"rec{qb}_{s}_{j}")
                    with nc.allow_low_precision(reason="softmax recip"):
                        nc.vector.reciprocal(rec[:], av[:, base + 64:base + 65])
                    asb = avsb.tile([128, 64], BF16, tag="asb",
                                    name=f"asb{qb}_{s}_{j}")
                    # Pool cannot read PSUM on trn2 -> divide on DVE
                    nc.vector.tensor_scalar(asb[:], av[:, base:base + 64],
                                            rec[:, 0:1], None,
                                            op0=AluOpType.mult)
                    stage2.append((s, j, asb))

                def finish_stage2(per_j=False):
                    # transpose divided scores tiles back to [dh, q] (PE)
                    flushed = set()
                    for s, j, asb in stage2:
                        flushed.add(j)
                        if per_j:
                            # tail mode: single-qtile transpose + copy so the
                            # final O-projection can start per 128 queries
                            avT1 = psJ.tile([64, 128], BF16, tag="pj",
                                            name=f"avT1{qb}_{s}_{j}")
                            nc.tensor.transpose(avT1[:], asb[:], eye[:])
                            qt0 = 4 * qb + j
                            nc.vector.tensor_copy(
                                at_all[s * 64:(s + 1) * 64,
                                       qt0 * KT:(qt0 + 1) * KT],
                                avT1[:])
                            continue
                        if j % 2 == 0:
                            avT[s] = psJ.tile([64, 256], BF16, tag="pj",
                                              name=f"avT{qb}_{s}_{j}")
                        nc.tensor.transpose(
                            avT[s][:, (j % 2) * 128:(j % 2) * 128 + 128],
                            asb[:], eye[:])
                        if j % 2 == 1:
                            qt0 = 4 * qb + j - 1
                            nc.vector.tensor_copy(
                                at_all[s * 64:(s + 1) * 64,
                                       qt0 * KT:(qt0 + 2) * KT],
                                avT[s][:])
                    stage2.clear()
                    return flushed

                def emit_avs_slot(ex_, ta_, s_):
                    for h_ in range(2):
                        t_ = ta_ + h_
                        for j_ in range(4):
                            qt_ = 4 * qb + j_
                            if t_ > qt_:
                                continue
                            # start=True clears has_written for the WHOLE
                            # PSUM bank on hw, so only the first matmul into
                            # this slot's bank may set it; later slices'
                            # first writes overwrite via the cleared bits
                            nc.tensor.matmul(
                                av[:, s_ * AVS + j_ * 65:
                                   s_ * AVS + j_ * 65 + 65],
                                ex_[:, h_ * QB + j_ * 128:
                                    h_ * QB + j_ * 128 + 128],
                                vp_all[:, t_ * VPW + s_ * 65:
                                       t_ * VPW + s_ * 65 + 65],
                                start=(t_ == 0 and j_ == 0),
                                stop=(t_ == qt_))

                def stage1_checks(ta_):
                    if ta_ + 1 >= 4 * qb:
                        for j_ in (ta_ - 4 * qb, ta_ + 1 - 4 * qb):
                            if 0 <= j_ < 4:
                                for s_ in slots:
                                    finish_stage1(s_, j_)

                def emit_avs(exs_, ta_):
                    for s_ in slots:
                        emit_avs_slot(exs_[s_], ta_, s_)
                    stage1_checks(ta_)

                def emit_logits(g, qhalf=None):
                    return emit_logits_for(qb, slots, g, qhalf, half_tiles)

                def emit_exp(g, lgs_g, qhalf=None):
                    ta = 2 * g
                    exs = {}
                    for s in slots:
                        lg, start_col = lgs_g[s]
                        if qhalf is None or qhalf[0] == 0:
                            ex = exp_pool.tile([128, 2 * QB], BF16, tag="ex",
                                               name=f"ex{qb}_{g}_{s}")
                            if qhalf is not None:
                                half_ex[s] = ex
                        else:
                            ex = half_ex[s]
                        if qhalf is not None:
                            q0, q1 = qhalf
                            # h=0 span [q0:q1]; h=1 valid span starts at 128
                            nc.scalar.activation(ex[:, q0:q1], lg[:, q0:q1],
                                                 AF.Exp, scale=0.125)
                            l1 = max(128, q0)
                            nc.scalar.activation(
                                ex[:, QB + l1:QB + q1],
                                lg[:, QB + l1:QB + q1],
                                AF.Exp, scale=0.125)
                            exs[s] = ex
                            continue
                        m1_ = ta + 1 - 4 * qb
                        if 0 < start_col and m1_ > 1:
                            # second diagonal group: skip the fully-masked
                            # [QB : QB+128*m1_) junk columns with a split exp
                            nc.scalar.activation(ex[:, start_col:QB],
                                                 lg[:, start_col:QB],
                                                 AF.Exp, scale=0.125)
                            nc.scalar.activation(ex[:, QB + 128 * m1_:2 * QB],
                                                 lg[:, QB + 128 * m1_:2 * QB],
                                                 AF.Exp, scale=0.125)
                        else:
                            nc.scalar.activation(ex[:, start_col:2 * QB],
                                                 lg[:, start_col:2 * QB],
                                                 AF.Exp, scale=0.125)
                        exs[s] = ex
                    return exs

                # Pipelined loop: logits for group g+1 are emitted right
                # after exp(g), so background/AV PE work can never delay the
                # logits feeding the next exp.  psL rotation gives exactly
                # the right backpressure (lg g+1 reuses the buf exp(g-ish)
                # reads, and that read precedes in program order).
                ngroups = nkt // 2
                prev = None
                half_tiles = {}
                half_ex = {}
                # per-qb PE slack credit (ns) available for deferred steps
                if qb >= QBLATE:
                    slack_per_group = SLACKL2 if len(slots) == 2 else SLACKL1
                else:
                    slack_per_group = SLACK2 if len(slots) == 2 else SLACK1
                credit = 0.0
                if pre is None:
                    force_need(qb - 0.5)  # only chunk-qb's Q gates group 0
                if qb == 0:
                    # front split: half-A logits+exp only need Q tokens
                    # [0:256) (rope piece a) — ACT starts ~1.5us sooner
                    la = emit_logits(0, qhalf=(0, 256))
                    emit_exp(0, la, qhalf=(0, 256))
                    pend = emit_logits(0, qhalf=(256, 512))
                else:
                    pend = pre if pre is not None else emit_logits(0)
                nxt = None
                for g in range(ngroups):
                    ta = 2 * g
                    if qb == 0 and g == 0:
                        exs = emit_exp(g, pend, qhalf=(256, 512))
                    else:
                        exs = emit_exp(g, pend)
                    pend = None
                    if g + 1 < ngroups:
                        force_need((2 * (g + 1) + 1) // 4)
                        pend = emit_logits(g + 1)
                    elif qb + 1 < NQB:
                        # cross-qb pipelining: next q-block's first logits
                        # are issued under this block's last exp, so the
                        # boundary (final AV/divides) never gates ACT
                        nxt = prologue_for(qb + 1)
                    # AV for the previous group's scores fills PE while ACT
                    # runs the exps.  Force any V-prep whose vp tiles these
                    # AV matmuls read (deadline c+0.75 at most).
                    if prev is not None:
                        tpair = prev[1] + 1
                        force_need(tpair // 4
                                   + (0.75 if tpair % 4 < 2 else 0.9))
                        for s in slots:
                            emit_avs_slot(prev[0][s], prev[1], s)
                    # spend PE slack on deferred steps (cost-budgeted)
                    credit += slack_per_group
                    urgent_used = False
                    while bg_steps:
                        best = min(range(len(bg_steps)),
                                   key=lambda i_: bg_steps[i_][0])
                        dl, cost, fn = bg_steps[best]
                        # at most ONE urgent (over-credit) pop per group so
                        # next-chunk prep never bursts into the exp stream
                        urgent = dl <= qb + URG and not urgent_used
                        if cost <= credit or urgent:
                            bg_steps.pop(best)
                            if SCHED_TRACE:
                                print(f"POP qb{qb} g{g} dl{dl} cost{cost} "
                                      f"urgent{urgent} credit{credit:.0f}")
                            fn()
                            credit -= cost
                            if cost > credit + cost:
                                pass
                            if urgent:
                                urgent_used = True
                                credit = max(credit, 0.0)
                        else:
                            break
                    fl = finish_stage2()
                    if 1 in fl and qb < NQB - 1:
                        # at_all qtiles 4qb..4qb+1 are final: first oproj
                        # half can start dripping within this q-block
                        queue_oproj(qb, hsel=0)
                        oproj_h0_queued.add(qb)
                    if prev is not None:
                        stage1_checks(prev[1])
                    prev = (exs, ta)
                force_need(qb + 0.95)
                emit_avs(*prev)
                if qb == NQB - 1:
                    # tail: interleave the final O-projection with the last
                    # qtiles' division/transpose chains, one qtile at a time
                    rest = list(stage2)
                    for j0 in range(4):
                        stage2[:] = [e for e in rest if e[1] == j0]
                        finish_stage2(per_j=True)
                        emit_oproj_span(qb, j0, j0 + 1, f"t{j0}")
                else:
                    finish_stage2()
                    if qb in oproj_h0_queued:
                        queue_oproj(qb, hsel=1)
                    else:
                        queue_oproj(qb)
                return nxt

            # ---------------- interleaved main loop ---------------------
            def queue_proj_chunk(c):
                """DMA the chunk now; queue K/Q/V proj as background steps."""
                c0, c1 = c * CH, (c + 1) * CH
                xc_all = xcp.tile([128, 6 * CH], BF16, tag="xc", name=f"xca{c}",
                                  bufs=5)
                cs_c = rtp.tile([128, 2 * CH], F16, tag="cosc", name=f"cs{c}",
                                bufs=4)
                cosf_c = cs_c[:, 0:CH]
                sins_c = cs_c[:, CH:2 * CH]
                xr = xc_all[:].rearrange("p (i c) -> p i c", c=CH)
                xtr = xt_d[:, c0:c1].rearrange("(i p) c -> p i c", p=128)
                csr = cs_c[:].rearrange("p (h s) -> p h s", h=2)
                csd = cossin_d[:].rearrange("p (h s) -> p h s", h=2)[:, :, c0:c1]
                if c == 0:
                    # front latency: land the first half-chunk, then wq (Q
                    # projection gate), then cos/sin, then the second half
                    hf = CH // 2
                    nc.sync.dma_start(xr[:, :, 0:hf], xtr[:, :, 0:hf])
                    nc.sync.dma_start(wq_all[:], wqt_d[:])
                    nc.sync.dma_start(csr, csd)
                    # eye+mask before x0b: the half-A logits' mask matmuls
                    # need it ~1us before K-b (x0b's consumer) is read
                    nc.sync.dma_start(em_all[:], eyemask_d[:])
                    nc.sync.dma_start(xr[:, :, hf:CH], xtr[:, :, hf:CH])
                else:
                    nc.sync.dma_start(xr, xtr)
                    nc.sync.dma_start(csr, csd)
                xc = [xc_all[:, i * CH:(i + 1) * CH] for i in range(6)]

                def rope_piece(w, dst, p0, p1):
                    # rope over token columns [c0+p0, c0+p1) of this chunk
                    pw = p1 - p0
                    ps = psJ.tile([128, pw], F32, tag="pj", name=f"pp{c}_{p0}")
                    for i in range(6):
                        nc.tensor.matmul(ps[:], w[i], xc[i][:, p0:p1],
                                         start=(i == 0), stop=(i == 5))
                    tsw = rtp.tile([128, pw], F32, tag="tsw")
                    nc.vector.stream_shuffle(tsw[:], ps[:], ROT16)
                    m1 = rtp.tile([128, pw], F32, tag="m1")
                    nc.vector.tensor_tensor(m1[:], ps[:], cosf_c[:, p0:p1],
                                            op=AluOpType.mult)
                    m2 = rtp.tile([128, pw], F32, tag="m2")
                    nc.gpsimd.tensor_tensor(m2[:], tsw[:], sins_c[:, p0:p1],
                                            op=AluOpType.mult)
                    # add on DVE: Pool's 2x-slower ops would serialize the
                    # rope tail on the critical path to the next logits
                    aeng = nc.vector if (ADD_DVE or c <= 1) else nc.gpsimd
                    aeng.tensor_tensor(dst[:, c0 + p0:c0 + p1],
                                       m1[:], m2[:], op=AluOpType.add)

                def rope_step(w, dst, pieces=1):
                    pw = CH // pieces
                    for p in range(pieces):
                        rope_piece(w, dst, p * pw, (p + 1) * pw)

                def v_tile_step(i_):
                    # V^T-direct: out [tok 128, dh 128] per token tile —
                    # stationary x slice gives token output partitions, so V
                    # lands already transposed for the AV moving operand (no
                    # separate PE transpose or vt staging copy)
                    t_ = 4 * c + i_
                    vtp = psJ.tile([128, 128], F32, tag="pj",
                                   name=f"vtp{c}_{i_}")
                    for i in range(6):
                        nc.tensor.matmul(
                            vtp[:], xc[i][:, i_ * 128:(i_ + 1) * 128],
                            wv_t[i], start=(i == 0), stop=(i == 5))
                    nc.vector.tensor_copy(
                        vp_all[:, t_ * VPW:t_ * VPW + 64], vtp[:, 0:64])
                    nc.vector.tensor_copy(
                        vp_all[:, t_ * VPW + 65:t_ * VPW + 129],
                        vtp[:, 64:128])

                pieces = 2
                pw = CH // pieces
                rp_cost = pw * 2.6
                if c == 0:
                    # qb0 g0 needs kt tiles 0-1 (K-a) + ALL of Q; K-b (tiles
                    # 2-3) is only read at g1 -> order K-a, Q-a, Q-b, K-b
                    order = [(wk_t, kt_rot, 0), (wq_t, qt_rot, 0)]
                    for p in range(1, pieces):
                        order.append((wq_t, qt_rot, p * pw))
                    for p in range(1, pieces):
                        order.append((wk_t, kt_rot, p * pw))
                    for w_, dst_, p0_ in order:
                        bg_steps.append((c, rp_cost,
                                         (lambda w2, d2, p0: lambda:
                                          rope_piece(w2, d2, p0, p0 + pw))
                                         (w_, dst_, p0_)))
                else:
                    # Q-rope gates qb c's FIRST logits group (K/V tiles of
                    # chunk c are only read late in qb c) -> Q first, with an
                    # earlier deadline so drips/forces prioritise it
                    for p in range(pieces):
                        bg_steps.append((c - 0.5, rp_cost, (lambda p0: lambda:
                                         rope_piece(wq_t, qt_rot, p0,
                                                    p0 + pw))(p * pw)))
                    for p in range(pieces):
                        bg_steps.append((c, rp_cost, (lambda p0: lambda:
                                         rope_piece(wk_t, kt_rot, p0,
                                                    p0 + pw))(p * pw)))
                # V tile i_ is first read by the AV pair covering k-tiles
                # 4c+2*(i_//2) -> deadline c+0.7 (first pair) / c+0.9 (second)
                for i_ in range(4):
                    vdl = (0.0 if c == 0 else c) + (0.7 if i_ < 2 else 0.85)
                    bg_steps.append((vdl, 380.0, (lambda ii: lambda:
                                     v_tile_step(ii))(i_)))

            def drain_bg():
                while bg_steps:
                    bg_steps.pop(0)[2]()

            queue_proj_chunk(0)
            init_consts_late()
            # drain chunk 0 now; later chunks prefetch 2 ahead and their
            # compute interleaves into the attention groups
            i0 = 0
            while i0 < len(bg_steps):
                if bg_steps[i0][0] <= 0:
                    bg_steps.pop(i0)[2]()
                else:
                    i0 += 1
            queue_proj_chunk(1)
            # wv/wo after chunk-1's x/cos DMAs: x1 gates qb1's Q-rope (ACT
            # critical path) while wv is first read ~1.5us later by V-prep
            nc.sync.dma_start(wvo_all[:], wvo_d[:])
            nxt_pre = None
            for qb in range(NQB):
                if qb == 0:
                    queue_proj_chunk(2)
                if qb + 3 < NCH:
                    queue_proj_chunk(qb + 3)
                nxt_pre = attention_qb(qb, [0] + ([1] if qb in bset else []),
                                       pre=nxt_pre)
            drain_bg()
            if DEBUG:
                nc.sync.dma_start(dbg_kt[:], kt_rot[:])
                nc.sync.dma_start(dbg_qt[:], qt_rot[:])
                nc.sync.dma_start(dbg_vp[:], vp_all[:])
                nc.sync.dma_start(dbg_at[:], at_all[:])

    nc.compile()
    return nc


def _get_program(bset):
    key = tuple(bset)
    if key not in _PROGRAMS:
        _PROGRAMS[key] = _build_program(key)
    return _PROGRAMS[key]


def _to_bf16(a):
    import ml_dtypes
    return np.asarray(a, np.float32).astype(ml_dtypes.bfloat16)


def _prep_core_inputs(core, x2d_T16, token_positions, Wq, Wk, Wv, Wo):
    hA, hB = CORE_HEADS[core]
    pos = token_positions.astype(np.float64)
    inv_freq = 1.0 / (THETA ** (np.arange(0, DH, 2, dtype=np.float64) / DH))  # [32]
    ang = pos[:, None] * inv_freq[None, :]          # [S, 32]
    cosv, sinv = np.cos(ang), np.sin(ang)           # [S, 32]

    cosf = np.empty((128, S), np.float16)
    sins = np.empty((128, S), np.float16)
    for r in range(64):
        q, i = r // 32, r % 32
        f = 16 * q + (i % 16)
        cosf[r] = cosf[r + 64] = cosv[:, f].astype(np.float16)
        sgn = -1.0 if i < 16 else 1.0
        sins[r] = sins[r + 64] = (sgn * sinv[:, f]).astype(np.float16)

    def _winterleave(wt):
        # [768, 128] -> [128, 6*128]: partition p holds rows {128i+p}
        return np.ascontiguousarray(
            wt.reshape(6, 128, 128).transpose(1, 0, 2).reshape(128, 768))

    rows = np.concatenate([hA * DH + PERM64, hB * DH + PERM64])
    wqt = _to_bf16(_winterleave(Wq[rows].T))   # [128,768]
    wkt = _to_bf16(_winterleave(Wk[rows].T))
    vrows = np.concatenate([np.arange(hA * DH, (hA + 1) * DH),
                            np.arange(hB * DH, (hB + 1) * DH)])
    wvt = _to_bf16(_winterleave(Wv[vrows].T))  # [128,768]
    wot = _to_bf16(np.ascontiguousarray(Wo[:, vrows].T))  # [128,768]

    # -200 (not -inf): exp(-200*0.125) ~ 1e-11 == 0 for our sums, and the
    # hardware ACT exp table NaNs on astronomically negative inputs
    maskm = np.where(np.arange(128)[None, :] >= np.arange(128)[:, None],
                     0.0, -200.0).astype(np.float32)  # [k', q']
    return {
        "xt": x2d_T16,
        "wkt": wkt,
        "wqt": wqt,
        "wvo": np.concatenate([wvt, wot], axis=1),
        "cossin": np.concatenate([cosf, sins], axis=1),
        "eyemask": np.concatenate(
            [_to_bf16(np.eye(128, dtype=np.float32)), _to_bf16(maskm)],
            axis=1),
    }


def _dispatch_group(nc, in_maps, devices):
    """Async-dispatch one program on a device subset; returns (arrs, names, avals, n)."""
    import jax
    from jax.sharding import Mesh, PartitionSpec
    from concourse import bass2jax, mybir

    bass2jax.install_neuronx_cc_hook()
    n = len(in_maps)
    partition_name = (nc.partition_id_tensor.name
                      if nc.partition_id_tensor else None)
    in_names, out_names, out_avals, zero_outs = [], [], [], []
    for alloc in nc.m.functions[0].allocations:
        if not isinstance(alloc, mybir.MemoryLocationSet):
            continue
        name = alloc.memorylocations[0].name
        if alloc.kind == "ExternalInput":
            if name != partition_name:
                in_names.append(name)
        elif alloc.kind == "ExternalOutput":
            shape = tuple(alloc.tensor_shape)
            dtype = mybir.dt.np(alloc.dtype)
            out_names.append(name)
            out_avals.append(jax.core.ShapedArray(shape, dtype))
            zero_outs.append(np.zeros(shape, dtype))
    n_params = len(in_names)
    all_names = in_names + out_names
    if partition_name is not None:
        all_names = all_names + [partition_name]
    donate = tuple(range(n_params, n_params + len(out_names)))

    def _body(*args):
        operands = list(args)
        if partition_name is not None:
            operands.append(bass2jax.partition_id_tensor())
        outs = bass2jax._bass_exec_p.bind(
            *operands, out_avals=tuple(out_avals), in_names=tuple(all_names),
            out_names=tuple(out_names), lowering_input_output_aliases=(),
            sim_require_finite=True, sim_require_nnan=True, nc=nc)
        return tuple(outs)

    try:
        from jax.experimental.shard_map import shard_map
    except ImportError:
        from jax.shard_map import shard_map  # newer jax

    mesh = Mesh(np.asarray(devices), ("core",))
    in_specs = (PartitionSpec("core"),) * (n_params + len(out_names))
    out_specs = (PartitionSpec("core"),) * len(out_names)
    sharded = jax.jit(
        shard_map(_body, mesh=mesh, in_specs=in_specs, out_specs=out_specs,
                  check_rep=False),
        donate_argnums=donate, keep_unused=True)
    per_core = [[np.asarray(m[nm]) for nm in in_names] for m in in_maps]
    concat_in = [np.concatenate([per_core[c][i] for c in range(n)], axis=0)
                 for i in range(n_params)]
    concat_zeros = [np.zeros((n * z.shape[0], *z.shape[1:]), z.dtype)
                    for z in zero_outs]
    out_arrs = sharded(*concat_in, *concat_zeros)
    return out_arrs, out_names, out_avals, n


def kernel(x, token_positions, Wq, Wk, Wv, Wo):
    import jax

    x = np.asarray(x)
    token_positions = np.asarray(token_positions)
    Wq, Wk, Wv, Wo = (np.asarray(a, np.float32) for a in (Wq, Wk, Wv, Wo))
    B = x.shape[0]
    assert x.shape == (B, S, D) and B == 1

    x2d_T16 = _to_bf16(np.ascontiguousarray(x[0].T))  # [768, 4096] bf16

    in_maps = [_prep_core_inputs(c, x2d_T16, token_positions, Wq, Wk, Wv, Wo)
               for c in range(8)]

    nc_even = _get_program(BSET_EVEN)
    nc_odd = _get_program(BSET_ODD)

    devs = jax.devices()
    # even program on devices 0-3 <- logical cores 0,2,4,6
    # odd  program on devices 4-7 <- logical cores 1,3,5,7
    g1_maps = [in_maps[c] for c in (0, 2, 4, 6)]
    g2_maps = [in_maps[c] for c in (1, 3, 5, 7)]

    arrs1, names1, avals1, n1 = _dispatch_group(nc_even, g1_maps, devs[0:4])
    arrs2, names2, avals2, n2 = _dispatch_group(nc_odd, g2_maps, devs[4:8])

    def collect(arrs, names, avals, n):
        res = []
        for c in range(n):
            res.append({
                nm: np.asarray(arrs[i]).reshape(n, *avals[i].shape)[c]
                for i, nm in enumerate(names)})
        return res

    res1 = collect(arrs1, names1, avals1, n1)
    res2 = collect(arrs2, names2, avals2, n2)

    acc = np.zeros((D, S), np.float32)
    for r in res1 + res2:
        acc += r["opart"].astype(np.float32)
    out = np.ascontiguousarray(acc.T).reshape(1, S, D)
    return out

